# revision 18
# baseline (speedup 1.0000x reference)
"""Trainium2 Bass kernel for EnergyIrrepModulation (fp16 pipeline).

Computes out[m, e, d] = x[m, d] * gates_full[e, d] where
gates = MLP(e_feat) : [nE, n_copies], expanded to [nE, D] via the static
irrep index map for IRREPS = [(64, 1), (32, 3), (16, 5)].

Sharding: data-parallel over M (4096 rows -> 512 rows per core, 8 cores).
Gates/MLP params are replicated; each core redundantly computes the tiny MLP.

The kernel is HBM-write-bound (full-size output), so everything runs in
fp16 (harness tolerance 2e-2; fp16 keeps max rel err ~1e-3):
  - output stores are fp16: 24.6 MB/core instead of 49.2 MB
  - DVE tensor_tensor multiplies hit the 2x perf mode (16-bit, step-1 APs)
  - PE matmuls are single-pass (no fp32 LOW/HIGH double pumping)

Per-core device plan:
  1. All MLP params arrive packed in ONE [128, 1080] fp16 tensor; the host
     pre-transposes e_feat so no on-device transposes are needed.
  2. Tiny MLP on the tensor engine (fp16 in, f32 PSUM); biases+ReLU fused
     on the scalar engine (b3 added along the free dim with a
     ones[100,1] @ b3[1,112] matmul).
  3. Gates are expanded 112 -> 240 on the e-partitions (3 small DVE
     broadcast-AP copies), flattened to partition 0 via SBUF->SBUF DMA,
     then broadcast to all 128 partitions via PE ones[1,128]-matmuls into
     PSUM (512-col bank pieces) copied out by ACT (chunk 0 by DVE, which
     is idle pre-loop).  partition_broadcast is deliberately NOT used: its
     Q7 SBUF traffic nearly stalls concurrent DVE work (measured 3.5us ->
     12.5us on an overlapped tensor_mul).
  4. Main loop: one fp16 tensor_mul per (e-chunk, m-tile) with the x
     operand stride-0-broadcast over e (2x DVE mode confirmed with the
     broadcast AP); whole-tile stores alternate between the two HWDGE
     rings (sync + scalar).
"""

import sys
from contextlib import ExitStack

import numpy as np

try:
    import concourse.bass as bass  # noqa: F401
except ImportError:  # pragma: no cover
    sys.path.insert(0, "/opt/trn_rl_repo")
    import concourse.bass as bass

import concourse.bacc as bacc
import concourse.tile as tile
from concourse import mybir
from concourse.bass_utils import run_bass_kernel_spmd

FP16 = mybir.dt.float16
FP32 = mybir.dt.float32

M, D = 4096, 240
NE, E_DIM, HIDDEN, NCOP = 100, 64, 256, 112
N_CORES = 8
MC = M // N_CORES          # 512 rows per core
MT = MC // 128             # 4 m-tiles of 128 rows

# uneven e-chunks: small first chunk -> first multiply/store starts early;
# small last chunk -> the final store (the tail) is short; 28-e body keeps
# the muls big (small muls pay ~0.3-0.4us fixed overhead each)
ECH = [8, 28, 28, 28, 8]
EOFF = [0, 8, 36, 64, 92]

# packed param layout (columns of the [128, NPARAM] fp16 tensor)
C_W2A, C_W2B = 0, 256
C_W3A, C_W3B = 512, 624
C_B1, C_B2 = 736, 738
C_W1 = 740                 # [64, 128] x 2 stacked on partition halves
C_ET = 868                 # e_featT [64, 100] duplicated on both halves
C_B3 = 968                 # [1, 112] on partition 0
NPARAM = 1080

_CACHE = {}


def _build_program():
    nc = bacc.Bacc(None, target_bir_lowering=False, debug=False)

    x_d = nc.dram_tensor("x", [MC, D], FP16, kind="ExternalInput")
    p_d = nc.dram_tensor("params", [128, NPARAM], FP16, kind="ExternalInput")
    out_d = nc.dram_tensor("out", [MC, NE * D], FP16, kind="ExternalOutput")

    with tile.TileContext(nc) as tc, ExitStack() as ctx:
        const_pool = ctx.enter_context(tc.tile_pool(name="const", bufs=1))
        mlp_pool = ctx.enter_context(tc.tile_pool(name="mlp", bufs=1))
        psum_mlp = ctx.enter_context(
            tc.tile_pool(name="psum_mlp", bufs=2, space="PSUM")
        )
        psum_bc = ctx.enter_context(
            tc.tile_pool(name="psum_bc", bufs=2, space="PSUM")
        )
        g_pool = ctx.enter_context(tc.tile_pool(name="gates", bufs=5))
        x_pool = ctx.enter_context(tc.tile_pool(name="xin", bufs=1))
        out_pool = ctx.enter_context(tc.tile_pool(name="out", bufs=5))

        p_t = const_pool.tile([128, NPARAM], FP16)
        # critical first-layer params (W1, eT, biases) land first
        nc.sync.dma_start(out=p_t[:, C_B1:NPARAM], in_=p_d[:, C_B1:NPARAM])
        nc.scalar.dma_start(out=p_t[:, 0:C_B1], in_=p_d[:, 0:C_B1])
        ones_t = const_pool.tile([1, 128], FP16)
        nc.vector.memset(ones_t[:], 1.0)

        # x for the whole core, loaded once: [128, MT*D], row p holds the
        # 4 m-tile rows t*128+p  (SWDGE so the HWDGE rings stay free)
        x_t = x_pool.tile([128, MT * D], FP16)
        nc.gpsimd.dma_start(
            out=x_t[:].rearrange("p (t d) -> p t d", d=D),
            in_=x_d[:].rearrange("(t p) d -> p t d", p=128),
        )

        relu = mybir.ActivationFunctionType.Relu

        # ---- MLP: h1T = relu(W1^T e_featT + b1), two [128, 100] tiles ----
        h1T = []
        for c in range(2):
            pl, ph = 64 * c, 64 * (c + 1)
            ps = psum_mlp.tile([128, NE], FP32)
            nc.tensor.matmul(
                ps[:], p_t[pl:ph, C_W1 : C_W1 + 128], p_t[pl:ph, C_ET : C_ET + NE],
                start=True, stop=True,
            )
            h = mlp_pool.tile([128, NE], FP16, tag=f"h1T{c}")
            nc.scalar.activation(h[:], ps[:], relu, bias=p_t[:, C_B1 + c : C_B1 + c + 1])
            h1T.append(h)

        # ---- h2T = relu(W2^T h1T + b2) ----
        h2T = []
        for c in range(2):
            ps = psum_mlp.tile([128, NE], FP32)
            nc.tensor.matmul(
                ps[:], p_t[:, C_W2A + c * 128 : C_W2A + (c + 1) * 128], h1T[0][:],
                start=True, stop=False,
            )
            nc.tensor.matmul(
                ps[:], p_t[:, C_W2B + c * 128 : C_W2B + (c + 1) * 128], h1T[1][:],
                start=False, stop=True,
            )
            h = mlp_pool.tile([128, NE], FP16, tag=f"h2T{c}")
            nc.scalar.activation(h[:], ps[:], relu, bias=p_t[:, C_B2 + c : C_B2 + c + 1])
            h2T.append(h)

        # ---- gates = h2 @ W3 + b3 : psum [100, 112], partition = e ----
        psg = psum_mlp.tile([NE, NCOP], FP32, tag="ps")
        nc.tensor.matmul(
            psg[:], h2T[0][:], p_t[:, C_W3A : C_W3A + NCOP], start=True, stop=False
        )
        nc.tensor.matmul(
            psg[:], h2T[1][:], p_t[:, C_W3B : C_W3B + NCOP], start=False, stop=False
        )
        # += ones[100,1] @ b3[1,112]: bias along the free dim via PE
        nc.tensor.matmul(
            psg[:], ones_t[0:1, 0:NE], p_t[0:1, C_B3 : C_B3 + NCOP],
            start=False, stop=True,
        )
        graw = mlp_pool.tile([NE, NCOP], FP16)
        nc.scalar.copy(graw[:], psg[:])

        # ---- expand 112 -> 240 on the e-partitions (fp16) ----
        gfull_e = mlp_pool.tile([NE, D], FP16)
        nc.vector.tensor_copy(gfull_e[:, 0:64], graw[:, 0:64])
        nc.vector.tensor_copy(
            gfull_e[:, 64:160].rearrange("e (i k) -> e i k", k=3),
            graw[:, 64:96].unsqueeze(2).to_broadcast((NE, 32, 3)),
        )
        nc.vector.tensor_copy(
            gfull_e[:, 160:240].rearrange("e (i k) -> e i k", k=5),
            graw[:, 96:112].unsqueeze(2).to_broadcast((NE, 16, 5)),
        )

        # ---- flatten to partition 0, then broadcast chunks to 128 parts ----
        # two flatten DMAs: chunk 0's slice first so its broadcast (and the
        # first multiply) is not gated on the full flatten
        c0n = ECH[0] * D
        flat = mlp_pool.tile([1, NE * D], FP16)
        nc.sync.dma_start(out=flat[:, 0:c0n], in_=gfull_e[0 : ECH[0], :])
        nc.sync.dma_start(out=flat[:, c0n:], in_=gfull_e[ECH[0] : NE, :])

        # PE ones-matmul broadcast: psum pieces [128, <=1120] (3 banks,
        # filled by <=512-wide bank-aligned matmuls) = ones^T @ flat slice,
        # copied to the chunk tile (DVE for chunk 0 — idle pre-loop; ACT
        # for the rest so DVE stays on the multiplies; wide pieces amortize
        # the ~400ns fixed ACT overhead).
        gchunks = []
        for ci, (sz, off) in enumerate(zip(ECH, EOFF)):
            g = g_pool.tile([128, sz * D], FP16, tag="g")
            n = sz * D
            pos = 0
            while pos < n:
                w = min(1536, n - pos)
                ps = psum_bc.tile([128, 1536], FP32, tag="bcp")
                q = 0
                while q < w:
                    wq = min(512, w - q)
                    nc.tensor.matmul(
                        ps[:, q : q + wq],
                        ones_t[:],
                        flat[0:1, off * D + pos + q : off * D + pos + q + wq],
                        start=True, stop=True,
                    )
                    q += wq
                if ci <= 1:
                    # chunks 0-1 on DVE (idle pre-loop; avoids waiting on
                    # the slower ACT copy chain for the second chunk)
                    nc.vector.tensor_copy(g[:, pos : pos + w], ps[:, 0:w])
                else:
                    nc.scalar.copy(g[:, pos : pos + w], ps[:, 0:w])
                pos += w
            gchunks.append(g)

        # ---- main loop: out[m, e, d] = x[m, d] * gates_full[e, d] ----
        si = 0
        for ci, (sz, off) in enumerate(zip(ECH, EOFF)):
            g_v = gchunks[ci][:].rearrange("p (e d) -> p e d", d=D)
            for mt in range(MT):
                x_v = (
                    x_t[:, mt * D : (mt + 1) * D]
                    .unsqueeze(1)
                    .to_broadcast((128, sz, D))
                )
                o_t = out_pool.tile([128, sz * D], FP16, tag="o")
                o_v = o_t[:].rearrange("p (e d) -> p e d", d=D)
                nc.vector.tensor_mul(o_v, x_v, g_v)
                eng = nc.sync if si % 2 == 0 else nc.scalar
                eng.dma_start(
                    out=out_d[
                        mt * 128 : (mt + 1) * 128, off * D : (off + sz) * D
                    ],
                    in_=o_t[:],
                )
                si += 1

    nc.compile()
    return nc


def _marshal(inputs):
    f16 = lambda a: np.ascontiguousarray(np.asarray(a, dtype=np.float16))
    x = f16(inputs["x"])
    W1, W2, W3 = f16(inputs["W1"]), f16(inputs["W2"]), f16(inputs["W3"])
    b1, b2, b3 = f16(inputs["b1"]), f16(inputs["b2"]), f16(inputs["b3"])
    eT = f16(np.asarray(inputs["e_feat"]).T)

    p = np.zeros((128, NPARAM), np.float16)
    p[:, C_W2A : C_W2A + 256] = W2[0:128]
    p[:, C_W2B : C_W2B + 256] = W2[128:256]
    p[:, C_W3A : C_W3A + NCOP] = W3[0:128]
    p[:, C_W3B : C_W3B + NCOP] = W3[128:256]
    p[:, C_B1] = b1[0:128]
    p[:, C_B1 + 1] = b1[128:256]
    p[:, C_B2] = b2[0:128]
    p[:, C_B2 + 1] = b2[128:256]
    p[0:64, C_W1 : C_W1 + 128] = W1[:, 0:128]
    p[64:128, C_W1 : C_W1 + 128] = W1[:, 128:256]
    p[0:64, C_ET : C_ET + NE] = eT
    p[64:128, C_ET : C_ET + NE] = eT
    p[0, C_B3 : C_B3 + NCOP] = b3

    return [
        {"x": x[i * MC : (i + 1) * MC], "params": p} for i in range(N_CORES)
    ]


def get_program():
    if "nc" not in _CACHE:
        _CACHE["nc"] = _build_program()
    return _CACHE["nc"]


def run(inputs, trace=False, **kwargs):
    """Run on 8 cores; returns (out [M, NE, D], BassKernelResults)."""
    nc = get_program()
    in_maps = _marshal(inputs)
    res = run_bass_kernel_spmd(
        nc, in_maps, core_ids=list(range(N_CORES)), trace=trace, **kwargs
    )
    out = np.concatenate(
        [
            np.asarray(res.results[i]["out"])
            .astype(np.float32)
            .reshape(MC, NE, D)
            for i in range(N_CORES)
        ],
        axis=0,
    )
    return out, res


def kernel(**inputs) -> np.ndarray:
    out, _ = run(inputs)
    return out
